# revision 9
# baseline (speedup 1.0000x reference)
"""
Trainium2 Bass kernel for nn_CausalSelfAttention_5214090298017.

Reference computes (B=2, T=2048, C=768, H=12, HD=64):
    q,k,v = split_heads(x @ W{q,k,v}.T + b)          # [B,H,T,HD]
    att   = softmax(mask(q @ k.T / sqrt(HD)))        # key-padding mask from attn_mask1
    y     = (att @ v).merge_heads() @ Wp.T + bp      # [B,T,C]

Sharding: 8 cores = 2 (batch) x 4 (head-groups of 3 heads).  Each core
computes a partial output  sum_{h in group} (att_h @ v_h) @ Wp_rows_h
([T, C]); the host sums the 4 group partials per batch (row-parallel Wp)
and concatenates over batch.

Device-side layout choices (per core):
  - Q^T, K^T stored [head_dim, T] so S^T = (K^T-tile).T-matmul gives
    score tiles [k_keys=128, q] with KEYS on partitions: the key-padding
    mask and the 1/sqrt(HD) scale are then applied for free by the Exp
    activation (per-partition bias + scalar scale).
  - V stored [T_k, 65] per head with a ones-column appended: the PV
    matmul accumulates [Y^T | softmax-denominator] in one pass.
  - Normalization (per-query 1/denom, which lives on the free axis of
    Y^T) is done by broadcasting 1/denom across partitions with a K=1
    matmul and one vector multiply.
  - Key compaction: only unmasked keys (~50%) are shipped/computed; the
    tail of the padded key range is killed by the same exp-bias mask.
All matmuls run as float32r (full-rate fp32 on the PE).
"""

import math
import os
import sys

import numpy as np

sys.path.insert(0, "/opt/trn_rl_repo")

import concourse.bass as bass  # noqa: E402
import concourse.tile as tile  # noqa: E402
from concourse import bacc, mybir  # noqa: E402
from concourse import bass_utils  # noqa: E402

F32 = mybir.dt.float32
F32R = mybir.dt.float32r

B, T, C, H = 2, 2048, 768, 12
HD = C // H          # 64
GROUPS = 4           # head-groups (tensor parallel)
HPG = H // GROUPS    # 3 heads per group
J = HPG * HD         # 192 local channels
NCORES = 8
SCALE = 1.0 / math.sqrt(HD)
MASK_NEG = -30000.0  # exp(-30000 + small) == 0.0 in fp32

COMPACT = os.environ.get("ATTN_NO_COMPACT", "") == ""


def _nchunks(n, cap=512, lo=256):
    """Split n (multiple of 128) into (start, width) chunks, each a multiple
    of 128 with lo <= width <= cap (so float32r matmuls stay full-rate)."""
    assert n % 128 == 0
    out = []
    pos = 0
    rem = n
    while rem > 0:
        w = min(cap, rem)
        if rem - w != 0 and rem - w < lo:
            # shrink this chunk so the tail stays >= lo
            w = rem - lo
            w = max(lo, (w // 128) * 128)
        out.append((pos, w))
        pos += w
        rem -= w
    assert all(w >= lo or n < lo for _, w in out)
    return out


def build_nc(tk, share_x):
    """Build the per-core Bass program.  tk = padded key length (mult of 128).
    share_x: key/value source x is the same tensor as the query x (dense mode)."""
    kk = tk // 128
    kq = T // 128

    nc = bacc.Bacc("TRN2", target_bir_lowering=False, debug=False)

    xt = nc.dram_tensor("xt", [769, T], F32R, kind="ExternalInput").ap()
    if share_x:
        xtkv = xt
    else:
        xtkv = nc.dram_tensor("xtkv", [769, tk], F32R, kind="ExternalInput").ap()
    wqT = nc.dram_tensor("wqT", [768, J], F32R, kind="ExternalInput").ap()
    wkT = nc.dram_tensor("wkT", [768, J], F32R, kind="ExternalInput").ap()
    wvT = nc.dram_tensor("wvT", [769, J], F32R, kind="ExternalInput").ap()
    bqv = nc.dram_tensor("bqv", [J], F32, kind="ExternalInput").ap()
    bkv = nc.dram_tensor("bkv", [J], F32, kind="ExternalInput").ap()
    mb = nc.dram_tensor("mb", [tk], F32, kind="ExternalInput").ap()
    # wpT rows = the group's 192 channels (+ bias row used with head 2)
    wpT = nc.dram_tensor("wpT", [J, 768], F32R, kind="ExternalInput").ap()
    bp4 = nc.dram_tensor("bp4", [768], F32R, kind="ExternalInput").ap()
    out = nc.dram_tensor("o", [T, 768], F32, kind="ExternalOutput").ap()

    from contextlib import ExitStack

    with tile.TileContext(nc) as tc, ExitStack() as ctx:
        const = ctx.enter_context(tc.tile_pool(name="const", bufs=1))
        ppool = ctx.enter_context(tc.tile_pool(name="psum", bufs=4, space="PSUM"))
        espool = ctx.enter_context(tc.tile_pool(name="es", bufs=4))
        mpool = ctx.enter_context(tc.tile_pool(name="misc", bufs=3))

        # ---------------- persistent SBUF tensors ----------------
        xt_s = const.tile([128, 6, T], F32R, tag="xt")
        xt1_s = const.tile([1, T], F32R, tag="xt1")
        if share_x:
            xkv_s, xkv1_s = xt_s, xt1_s
        else:
            xkv_s = const.tile([128, 6, tk], F32R, tag="xkv")
            xkv1_s = const.tile([1, tk], F32R, tag="xkv1")
        wq_s = const.tile([128, 6, J], F32R, tag="wq")
        wk_s = const.tile([128, 6, J], F32R, tag="wk")
        wv_s = const.tile([128, 6, J], F32R, tag="wv")
        wv1_s = const.tile([1, J], F32R, tag="wv1")
        bq_s = const.tile([128, 2], F32, tag="bq")
        bk_s = const.tile([128, 2], F32, tag="bk")
        mb_s = const.tile([128, kk], F32, tag="mb")
        wph_s = [const.tile([65 if h == 2 else 64, 768], F32R, tag=f"wp{h}", name=f"wp{h}") for h in range(3)]
        ones_s = const.tile([65, 128], F32R, tag="ones")
        qt_s = [const.tile([64, T], F32R, tag=f"qt{h}", name=f"qt{h}") for h in range(3)]
        kt_s = [const.tile([64, tk], F32R, tag=f"kt{h}", name=f"kt{h}") for h in range(3)]
        v_s = [const.tile([128, kk, 65], F32R, tag=f"v{h}", name=f"v{h}") for h in range(3)]
        yn_s = [const.tile([65 if h == 2 else 64, T], F32R, tag=f"yn{h}", name=f"yn{h}") for h in range(3)]

        # ---------------- input DMAs ----------------
        for ci in range(6):
            nc.sync.dma_start(xt_s[:, ci, :], xt[ci * 128:(ci + 1) * 128, :])
        nc.sync.dma_start(xt1_s[:, :], xt[768:769, :])
        if not share_x:
            for ci in range(6):
                nc.sync.dma_start(xkv_s[:, ci, :], xtkv[ci * 128:(ci + 1) * 128, :])
            nc.sync.dma_start(xkv1_s[:, :], xtkv[768:769, :])
        for w_s, wT in ((wq_s, wqT), (wk_s, wkT), (wv_s, wvT)):
            for ci in range(6):
                nc.sync.dma_start(w_s[:, ci, :], wT[ci * 128:(ci + 1) * 128, :])
        nc.sync.dma_start(wv1_s[:, :], wvT[768:769, :])
        nc.sync.dma_start(bq_s[:, 0:1], bqv[0:128][:, None])
        nc.sync.dma_start(bq_s[0:64, 1:2], bqv[128:192][:, None])
        nc.sync.dma_start(bk_s[:, 0:1], bkv[0:128][:, None])
        nc.sync.dma_start(bk_s[0:64, 1:2], bkv[128:192][:, None])
        nc.sync.dma_start(mb_s[:, :], mb.rearrange("(o p) -> p o", p=128))
        for h in range(3):
            nc.sync.dma_start(wph_s[h][0:64, :], wpT[h * 64:(h + 1) * 64, :])
        nc.sync.dma_start(wph_s[2][64:65, :], bp4[None, :])
        ONE_BITS = 0x3F800000  # 1.0f; memset can't take float32r directly
        U32 = mybir.dt.uint32
        nc.vector.memset(ones_s[:, :].bitcast(U32), ONE_BITS)
        nc.vector.memset(yn_s[2][64:65, :].bitcast(U32), ONE_BITS)
        for h in range(3):
            nc.vector.memset(v_s[h][:, :, 64:65].bitcast(U32), ONE_BITS)

        # ---------------- phase 1: projections ----------------
        # Q^T / K^T : out[j, t] = sum_c W.T[c, j] * x^T[c, t]   (j on partitions)
        for (w_s, x_src, b_s, dst, tlen) in (
            (wq_s, xt_s, bq_s, qt_s, T),
            (wk_s, xkv_s, bk_s, kt_s, tk),
        ):
            for blk, m in ((0, 128), (1, 64)):
                for (n0, nw) in _nchunks(tlen):
                    pt = ppool.tile([128, 1024], F32, tag="ps")
                    for ci in range(6):
                        nc.tensor.matmul(
                            pt[0:m, 0:nw],
                            lhsT=(w_s[:, ci, blk * 128:blk * 128 + m]),
                            rhs=(x_src[:, ci, n0:n0 + nw]),
                            start=(ci == 0),
                            stop=(ci == 5),
                        )
                    # psum -> sbuf with per-partition bias add (j on partitions)
                    if blk == 0:
                        nc.vector.tensor_scalar(
                            out=dst[0][:, n0:n0 + nw], in0=pt[0:64, 0:nw],
                            scalar1=b_s[0:64, 0:1], scalar2=None,
                            op0=mybir.AluOpType.add)
                        nc.vector.tensor_scalar(
                            out=dst[1][:, n0:n0 + nw], in0=pt[64:128, 0:nw],
                            scalar1=b_s[64:128, 0:1], scalar2=None,
                            op0=mybir.AluOpType.add)
                    else:
                        nc.vector.tensor_scalar(
                            out=dst[2][:, n0:n0 + nw], in0=pt[0:64, 0:nw],
                            scalar1=b_s[0:64, 1:2], scalar2=None,
                            op0=mybir.AluOpType.add)

        # V : out[t_k, j] = sum_c x_kv^T[c, t-tile].T * Wv.T[c, j]  (+ bias row)
        for tt in range(kk):
            pt = ppool.tile([128, 1024], F32, tag="ps")
            for ci in range(6):
                nc.tensor.matmul(
                    pt[:, 0:J],
                    lhsT=(xkv_s[:, ci, tt * 128:(tt + 1) * 128]),
                    rhs=(wv_s[:, ci, :]),
                    start=(ci == 0), stop=False)
            nc.tensor.matmul(
                pt[:, 0:J],
                lhsT=(xkv1_s[0:1, tt * 128:(tt + 1) * 128]),
                rhs=(wv1_s[0:1, :]),
                start=False, stop=True)
            for h in range(3):
                nc.vector.tensor_copy(v_s[h][:, tt, 0:64], pt[:, h * 64:(h + 1) * 64])

        # ---------------- phase 2: attention ----------------
        QCW = 1024
        for qc in range(T // QCW):
            q0 = qc * QCW
            for grp in ((0, 1), (2,)):
                yps = {}
                for h in grp:
                    yps[h] = ppool.tile([128, 1024], F32, tag="ps", name=f"yp{h}")
                for kt in range(kk):
                    sts = {}
                    for h in grp:
                        st = ppool.tile([128, 1024], F32, tag="ps", name=f"st{h}")
                        sts[h] = st
                        ktt = kt_s[h]
                        qtt = qt_s[h]
                        for nn in range(QCW // 512):
                            nc.tensor.matmul(
                                st[:, nn * 512:(nn + 1) * 512],
                                lhsT=(ktt[:, kt * 128:(kt + 1) * 128]),
                                rhs=(qtt[:, q0 + nn * 512:q0 + (nn + 1) * 512]),
                                start=True, stop=True)
                    for h in grp:
                        es = espool.tile([128, QCW], F32R, tag="es")
                        nc.scalar.activation(
                            out=es[:, :], in_=sts[h][:, :],
                            func=mybir.ActivationFunctionType.Exp,
                            bias=mb_s[:, kt:kt + 1], scale=SCALE)
                        for nn in range(QCW // 512):
                            nc.tensor.matmul(
                                yps[h][0:65, nn * 512:(nn + 1) * 512],
                                lhsT=(v_s[h][:, kt, :]),
                                rhs=(es[:, nn * 512:(nn + 1) * 512]),
                                start=(kt == 0), stop=(kt == kk - 1))
                # normalize: yn = Y^T * broadcast(1 / denom)
                for h in grp:
                    rd = mpool.tile([65, QCW], F32R, tag="rd")
                    with nc.allow_low_precision(reason="recip consumed as f32r"):
                        nc.vector.reciprocal(rd[64:65, :], yps[h][64:65, 0:QCW])
                    bc = ppool.tile([128, 1024], F32, tag="ps")
                    for nn in range(QCW // 512):
                        nc.tensor.matmul(
                            bc[0:64, nn * 512:(nn + 1) * 512],
                            lhsT=(ones_s[64:65, 0:64]),
                            rhs=(rd[64:65, nn * 512:(nn + 1) * 512]),
                            start=True, stop=True)
                    bcs = mpool.tile([65, QCW], F32R, tag="rd")
                    nc.vector.tensor_copy(bcs[0:64, :], bc[0:64, 0:QCW])
                    nc.vector.tensor_tensor(
                        out=yn_s[h][0:64, q0:q0 + QCW],
                        in0=yps[h][0:64, 0:QCW],
                        in1=bcs[0:64, :],
                        op=mybir.AluOpType.mult)

        # ---------------- phase 3: output projection ----------------
        # o[t, c] = sum_h yn_h^T[j, t].T @ wpT_h[j, c]   (+ ones-row * bp/4)
        for tt in range(kq):
            op = ppool.tile([128, 1024], F32, tag="ps")
            for (n0, nw) in _nchunks(768):
                for h in range(3):
                    m = 65 if h == 2 else 64
                    nc.tensor.matmul(
                        op[:, n0:n0 + nw],
                        lhsT=(yn_s[h][0:m, tt * 128:(tt + 1) * 128]),
                        rhs=(wph_s[h][0:m, n0:n0 + nw]),
                        start=(h == 0), stop=(h == 2))
            o_sb = espool.tile([128, 1024], F32, tag="es", name="o_sb")
            nc.vector.tensor_copy(o_sb[:, 0:768], op[:, 0:768])
            nc.sync.dma_start(out[tt * 128:(tt + 1) * 128, :], o_sb[:, 0:768])

    nc.compile()
    return nc


def _prep_core_inputs(x, attn_mask1, Wq, bq, Wk, bk, Wv, bv, Wp, bp):
    """Host-side sharding: returns (in_maps, tk, share_x)."""
    x = np.asarray(x, np.float32)
    attn_mask1 = np.asarray(attn_mask1)
    Wq, Wk, Wv, Wp = (np.asarray(a, np.float32) for a in (Wq, Wk, Wv, Wp))
    bq, bk, bv, bp = (np.asarray(a, np.float32) for a in (bq, bk, bv, bp))

    ones = np.ones((1, T), np.float32)
    xts = []
    for b in range(B):
        xts.append(np.concatenate([x[b].T, ones], axis=0))  # [769, T]

    if COMPACT:
        idxs = [np.nonzero(attn_mask1[b] != 0)[0] for b in range(B)]
        nmax = max(max(len(i) for i in idxs), 1)
        tk = ((nmax + 127) // 128) * 128
        share_x = False
        xkvs, mbs = [], []
        for b in range(B):
            idx = idxs[b]
            xg = np.zeros((tk, C), np.float32)
            xg[:len(idx)] = x[b][idx]
            row = np.ones((1, tk), np.float32)
            xkvs.append(np.concatenate([xg.T, row], axis=0))
            m = np.zeros(tk, np.float32)
            m[len(idx):] = MASK_NEG
            mbs.append(m)
    else:
        tk = T
        share_x = True
        xkvs = [None, None]
        mbs = [np.where(attn_mask1[b] != 0, 0.0, MASK_NEG).astype(np.float32)
               for b in range(B)]

    WqT, WkT, WvT, WpT = Wq.T.copy(), Wk.T.copy(), Wv.T.copy(), Wp.T.copy()

    in_maps = []
    for c in range(NCORES):
        b, g = c // GROUPS, c % GROUPS
        js = slice(g * J, (g + 1) * J)
        m = {
            "xt": xts[b],
            "wqT": np.ascontiguousarray(WqT[:, js]),
            "wkT": np.ascontiguousarray(WkT[:, js]),
            "wvT": np.concatenate([WvT[:, js], bv[js][None, :]], axis=0),
            "bqv": np.ascontiguousarray(bq[js]),
            "bkv": np.ascontiguousarray(bk[js]),
            "mb": mbs[b],
            "wpT": np.ascontiguousarray(WpT[js, :]),
            "bp4": (bp / GROUPS).astype(np.float32),
        }
        if not share_x:
            m["xtkv"] = xkvs[b]
        in_maps.append(m)
    return in_maps, tk, share_x


_CACHE = {}


def kernel(**inputs):
    in_maps, tk, share_x = _prep_core_inputs(**inputs)
    key = (tk, share_x)
    if key not in _CACHE:
        _CACHE[key] = build_nc(tk, share_x)
    nc = _CACHE[key]
    res = bass_utils.run_bass_kernel_spmd(nc, in_maps, list(range(NCORES)))
    out = np.zeros((B, T, C), np.float32)
    for c in range(NCORES):
        out[c // GROUPS] += res.results[c]["o"]
    return out


if __name__ == "__main__":
    # smoke test with random data (no reference available here)
    rng = np.random.default_rng(0)
    ins = {
        "x": rng.standard_normal((B, T, C), dtype=np.float32),
        "attn_mask1": rng.integers(0, 2, size=(B, T)).astype(np.int32),
        "Wq": (rng.standard_normal((C, C), dtype=np.float32) * 0.02),
        "bq": np.zeros(C, np.float32),
        "Wk": (rng.standard_normal((C, C), dtype=np.float32) * 0.02),
        "bk": np.zeros(C, np.float32),
        "Wv": (rng.standard_normal((C, C), dtype=np.float32) * 0.02),
        "bv": np.zeros(C, np.float32),
        "Wp": (rng.standard_normal((C, C), dtype=np.float32) * 0.02),
        "bp": np.zeros(C, np.float32),
    }
    out = kernel(**ins)
    print(out.shape, out.dtype, np.abs(out).max())
